# revision 8
# baseline (speedup 1.0000x reference)
"""Trainium2 Bass kernel for nn_CrossAttention (B=8, N1=64, N2=4096, C=768, H=12).

Strategy: data-parallel over batch across 8 NeuronCores (one item per core,
no collectives). All activations kept transposed (channels on partitions,
tokens on the free dim) so every matmul contracts over SBUF partitions.

Key algebraic restructurings (exploiting that the module's combine with v is
ELEMENTWISE, attn_t * v, not attn @ v):

  1. scores_h = q_h @ k_h^T = (q_h @ W_k_h) @ yT = A_h @ yT.  A = qT^T @ W_k
     is a tiny [768,768] precompute; scores then contract over the full
     K=128 partitions with the SAME moving operand (yT chunks) as the
     v-projection — k is never materialized.
  2. softmax normalization is deferred: U_h = exp(s_h) * vT_h is accumulated
     unnormalized; row-sums S come free via ACT's fused accum_out; 1/S is
     folded into the projection weights (O(C^2), not O(C*N2)).

Matmuls in bf16 (f32 PSUM accumulation), N=1024 moving; softmax stats f32.
Host pre-transposes/casts shards; HW output is outT f32 per core.
"""

import numpy as np
import ml_dtypes

import concourse.bass as bass
import concourse.mybir as mybir
import concourse.tile as tile
from concourse import bacc
from concourse.bass_utils import run_bass_kernel_spmd

BF16 = mybir.dt.bfloat16
F32 = mybir.dt.float32

B, N1, N2, C, H = 8, 64, 4096, 768, 12
HD = C // H              # 64
SCALE = HD ** -0.5       # 1/8
CT = C // 128            # 6 partition tiles of channels
CHUNK = 1024             # tokens per streamed chunk (bf16 moving max)
NCH = N2 // CHUNK        # 4 chunks
PAIRS = CT               # 6 head pairs (2 heads per 128-partition tile)

_CACHE = {}


def _build():
    nc = bacc.Bacc("TRN2", target_bir_lowering=False, debug=False)

    xT_d = nc.dram_tensor("xT", [C, N1], BF16, kind="ExternalInput")
    yT_d = nc.dram_tensor("yT", [C, N2], BF16, kind="ExternalInput")
    # wqvT: [768, 1536] = [W_q^T | W_v^T]  (k handled via A, never projected)
    wqvT_d = nc.dram_tensor("wqvT", [C, 2 * C], BF16, kind="ExternalInput")
    # wk: natural layout [c_out, c_in] = W_qkv[C:2C, :]
    wk_d = nc.dram_tensor("wk", [C, C], BF16, kind="ExternalInput")
    wprojT_d = nc.dram_tensor("wprojT", [C, C], BF16, kind="ExternalInput")
    bproj_d = nc.dram_tensor("bproj", [C, 1], F32, kind="ExternalInput")
    outT_d = nc.dram_tensor("outT", [C, N2], F32, kind="ExternalOutput")

    with tile.TileContext(nc) as tc:
        with (
            tc.tile_pool(name="persist", bufs=1) as pp,
            tc.tile_pool(name="work", bufs=2) as wp,
            tc.tile_pool(name="psum", bufs=2, space=bass.MemorySpace.PSUM) as psp,
        ):
            # ---- persistent tiles -------------------------------------------
            wqv_sb = [pp.tile([128, 2 * C], BF16, name=f"wqv{kk}", tag=f"wqv{kk}")
                      for kk in range(CT)]
            wk_sb = [pp.tile([128, C], BF16, name=f"wk{j}", tag=f"wk{j}")
                     for j in range(CT)]
            wp_sb = [pp.tile([128, C], BF16, name=f"wpr{kk}", tag=f"wpr{kk}")
                     for kk in range(CT)]
            wps_sb = [pp.tile([128, C], BF16, name=f"wps{kk}", tag=f"wps{kk}")
                      for kk in range(CT)]
            A_sb = [pp.tile([128, C], BF16, name=f"A{kk}", tag=f"A{kk}")
                    for kk in range(CT)]
            xT_sb = [pp.tile([128, N1], BF16, name=f"xT{kk}", tag=f"xT{kk}")
                     for kk in range(CT)]
            bias_sb = [pp.tile([128, 1], F32, name=f"bias{m}", tag=f"bias{m}")
                       for m in range(CT)]
            # block-diagonal q: qbd[0:64, g, 0:64] = qT head 2g,
            # qbd[64:128, g, 64:128] = qT head 2g+1, zeros elsewhere.
            # Lets A-prep contract K=128 in one clean full-array matmul.
            qbd = pp.tile([128, CT, 128], BF16, name="qbd", tag="qbd")
            U_sb = [pp.tile([128, N2], BF16, name=f"U{g}", tag=f"U{g}")
                    for g in range(PAIRS)]
            S_parts = [pp.tile([128, NCH], F32, name=f"Sp{g}", tag=f"Sp{g}")
                       for g in range(PAIRS)]
            zbias = pp.tile([128, 1], F32, name="zbias", tag="zbias")
            nc.gpsimd.memset(zbias[:], 0.0)
            nc.gpsimd.memset(qbd[:], 0.0)

            for kk in range(CT):
                row = slice(128 * kk, 128 * (kk + 1))
                nc.sync.dma_start(wqv_sb[kk][:], wqvT_d[row, :])
                nc.sync.dma_start(wk_sb[kk][:], wk_d[row, :])
                nc.sync.dma_start(wp_sb[kk][:], wprojT_d[row, :])
                nc.sync.dma_start(xT_sb[kk][:], xT_d[row, :])
                nc.sync.dma_start(bias_sb[kk][:], bproj_d[row, :])

            # ---- qT = (W_q @ xT) * scale ------------------------------------
            for m in range(CT):
                psq = psp.tile([128, N1], F32, name="psq", tag="pss", bufs=2)
                for kk in range(CT):
                    nc.tensor.matmul(
                        psq[:],
                        wqv_sb[kk][:, 128 * m:128 * (m + 1)],
                        xT_sb[kk][:],
                        start=(kk == 0), stop=(kk == CT - 1),
                    )
                nc.scalar.activation(qbd[0:64, m, 0:64], psq[0:64, :],
                                     mybir.ActivationFunctionType.Copy,
                                     bias=0.0, scale=SCALE)
                nc.scalar.activation(qbd[64:128, m, 64:128], psq[64:128, :],
                                     mybir.ActivationFunctionType.Copy,
                                     bias=0.0, scale=SCALE)

            # ---- A_h = q_h @ W_k_h  (scores = A @ yT later) -----------------
            # A^T tile (kk, pair g) = wk_pair_g^T @ qbd_g: K=128 contraction
            # thanks to the block-diagonal zero padding of qbd.
            for kk in range(CT):
                psA = psp.tile([128, C], F32, name="psA", tag="pskv", bufs=2)
                for g in range(PAIRS):
                    nc.tensor.matmul(
                        psA[:, 128 * g:128 * (g + 1)],
                        wk_sb[g][:, 128 * kk:128 * (kk + 1)],
                        qbd[:, g, :],
                        start=True, stop=True,
                    )
                if kk % 2 == 0:
                    nc.scalar.copy(A_sb[kk][:], psA[:])
                else:
                    nc.vector.tensor_copy(A_sb[kk][:], psA[:])

            # ---- stream over token chunks -----------------------------------
            for c in range(NCH):
                tok = slice(CHUNK * c, CHUNK * (c + 1))
                yT_c = [wp.tile([128, CHUNK], BF16, name=f"yTc{kk}", tag=f"yTc{kk}",
                                bufs=2) for kk in range(CT)]
                for kk in range(CT):
                    nc.sync.dma_start(yT_c[kk][:], yT_d[128 * kk:128 * (kk + 1), tok])

                # v projection: vT = W_v @ yT
                vT_c = [wp.tile([128, CHUNK], BF16, name=f"vTc{m}", tag=f"vTc{m}",
                                bufs=2) for m in range(CT)]
                for m in range(CT):
                    pskv = psp.tile([128, CHUNK], F32, name="pskv", tag="pskv", bufs=2)
                    for kk in range(CT):
                        for hf in range(2):  # same lhsT twice: LDW amortized
                            nc.tensor.matmul(
                                pskv[:, 512 * hf:512 * (hf + 1)],
                                wqv_sb[kk][:, C + 128 * m:C + 128 * (m + 1)],
                                yT_c[kk][:, 512 * hf:512 * (hf + 1)],
                                start=(kk == 0), stop=(kk == CT - 1),
                            )
                    if m % 2 == 0:
                        nc.scalar.copy(vT_c[m][:], pskv[:])
                    else:
                        nc.vector.tensor_copy(vT_c[m][:], pskv[:])

                # scores + softmax + combine, per head pair
                for g in range(PAIRS):
                    pss = psp.tile([128, CHUNK], F32, name="pss", tag="pss", bufs=2)
                    for kk in range(CT):
                        for hf in range(2):
                            nc.tensor.matmul(
                                pss[:, 512 * hf:512 * (hf + 1)],
                                A_sb[kk][:, 128 * g:128 * (g + 1)],
                                yT_c[kk][:, 512 * hf:512 * (hf + 1)],
                                start=(kk == 0), stop=(kk == CT - 1),
                            )
                    e_sb = wp.tile([128, CHUNK], BF16, name="e_sb", tag="e_sb", bufs=3)
                    nc.scalar.activation(e_sb[:], pss[:],
                                         mybir.ActivationFunctionType.Exp,
                                         bias=zbias[:], scale=1.0,
                                         accum_out=S_parts[g][:, c:c + 1])
                    nc.vector.tensor_mul(U_sb[g][:, tok], e_sb[:], vT_c[g][:])

            # ---- fold 1/S into projection weights ---------------------------
            for g in range(PAIRS):
                S_tot = wp.tile([128, 1], F32, name="S_tot", tag="S_tot", bufs=2)
                nc.vector.tensor_reduce(S_tot[:], S_parts[g][:],
                                        axis=mybir.AxisListType.X,
                                        op=mybir.AluOpType.add)
                R_g = wp.tile([128, 1], F32, name="R_g", tag="R_g", bufs=2)
                nc.vector.reciprocal(R_g[:], S_tot[:])
                nc.vector.tensor_scalar_mul(wps_sb[g][:], wp_sb[g][:], R_g[:])

            # ---- outT = W_proj_scaled @ U + b -------------------------------
            for m in range(CT):
                for n in range(NCH):
                    tok = slice(CHUNK * n, CHUNK * (n + 1))
                    psq2 = psp.tile([128, CHUNK], F32, name="psq2", tag="pskv", bufs=2)
                    for kk in range(CT):
                        for hf in range(2):
                            nc.tensor.matmul(
                                psq2[:, 512 * hf:512 * (hf + 1)],
                                wps_sb[kk][:, 128 * m:128 * (m + 1)],
                                U_sb[kk][:, CHUNK * n + 512 * hf:
                                           CHUNK * n + 512 * (hf + 1)],
                                start=(kk == 0), stop=(kk == CT - 1),
                            )
                    outc = wp.tile([128, CHUNK], F32, name="outc", tag="outc", bufs=2)
                    if n % 2 == 0:
                        nc.scalar.add(outc[:], psq2[:], add=bias_sb[m][:])
                    else:
                        nc.vector.tensor_scalar_add(outc[:], psq2[:], bias_sb[m][:])
                    nc.sync.dma_start(outT_d[128 * m:128 * (m + 1), tok], outc[:])

    nc.compile()
    return nc


def kernel(x, y, W_qkv, W_proj, b_proj):
    if "nc" not in _CACHE:
        _CACHE["nc"] = _build()
    nc = _CACHE["nc"]
    in_maps = make_in_maps(x, y, W_qkv, W_proj, b_proj)
    res = run_bass_kernel_spmd(nc, in_maps, core_ids=list(range(B)))
    out = np.empty((B, N2, C), np.float32)
    for i in range(B):
        out[i] = res.results[i]["outT"].T
    return out


def make_in_maps(x, y, W_qkv, W_proj, b_proj):
    bf = ml_dtypes.bfloat16
    W_qkv = np.asarray(W_qkv, np.float32)
    wqvT = np.ascontiguousarray(
        np.concatenate([W_qkv[:C].T, W_qkv[2 * C:].T], axis=1)).astype(bf)
    wk = np.ascontiguousarray(W_qkv[C:2 * C]).astype(bf)
    wprojT = np.ascontiguousarray(np.asarray(W_proj, np.float32).T).astype(bf)
    bproj = np.asarray(b_proj, np.float32).reshape(C, 1)

    in_maps = []
    for i in range(B):
        in_maps.append({
            "xT": np.ascontiguousarray(np.asarray(x[i], np.float32).T).astype(bf),
            "yT": np.ascontiguousarray(np.asarray(y[i], np.float32).T).astype(bf),
            "wqvT": wqvT,
            "wk": wk,
            "wprojT": wprojT,
            "bproj": bproj,
        })
    return in_maps
